# revision 57
# baseline (speedup 1.0000x reference)
"""Trainium2 Bass kernel for fused multi-head causal attention.

Module: out = o_proj(causal_attention(rope_swapped(qkv_proj(x)))).
Shapes: x [2, 2048, 2048], 16 heads, head_dim 128.

Sharding (8 cores): batch (2) x head-group (4 groups of 4 heads).
Each core computes qkv projection + attention for its 4 heads of its
batch, then a partial o_proj against its slice of w_o rows.  The
all-reduce after o_proj is done host-side by summing the 4 partials
per batch (mathematically identical, avoids device collectives).

Device-kernel design notes:
 - QKV and o_proj matmuls run in fp8-e4m3 DoubleRow perf mode (0.5
   cycles/row on PE) with hi+lo error compensation: a ~= hi(a) + lo(a)
   with both parts fp8; a@b ~= ah@bh + al@bh + ah@bl (lo x lo dropped,
   ~0.1% error).  3 chains x half-rate = 0.75x the bf16 PE cost.
   Weights are scaled by 64 host-side so their fp8 parts stay normal;
   the 1/64 is folded into the rope tables / V copy / output copy.
 - DoubleRow contracts 2 k-chunks of 128 per instruction; operands are
   laid out [128 partitions, 2, free] with the paired k-chunks adjacent
   in the free dim (prepared host-side).
 - The qkv bias is added with a single fp8 DoubleRow rank-1 matmul per
   output tile: lhsT [1, 2, 128] = (bias_hi | bias_lo), rhs = ones.
 - QKV projection computes Q^T/K^T/V^T ([head_dim, S] layout) directly:
   out = W_slice.T @ x^T, so attention's QK^T matmul needs no transposes.
 - Q/K head pairs are interleaved ([q_ha_lo | q_hb_lo] on 128 partitions)
   so RoPE's rotate_half partner lives at the SAME partition of a sibling
   tile -> full-width DVE ops (3 ops/element, no cross-partition shuffle).
 - Scores are computed transposed ([sk, sq]) so the PV matmul consumes
   exp(scores) directly; softmax denominators come from a ones-vector
   matmul; normalization is folded into the PSUM->SBUF copy of attn^T,
   which also emits the fp8 hi/lo pair layout for the o_proj chains.
 - Causal masking: fully-masked columns of diagonal score tiles are
   skipped outright; the remaining width-128 triangle is accumulated
   into the logits psum ON THE PE (cmask^T @ ident) so no vector-engine
   hop sits between QK^T and the exp.
 - V^T -> V transposes go through the DMA xbar (dma_start_transpose),
   off the PE/psum entirely; V projection runs as two 4-acc passes so
   its psum banks drain while the second pass computes.
 - Attention core (QK^T, exp, PV, denominators) stays bf16: a single
   uncompensated fp8 tensor there would inject ~3.6% output error
   (the hi/lo compensation only pays at contraction >= 256).
 - Output rows are assembled [128, 2048] f16 in SBUF and written with
   one wide DMA per s-tile (halves output bytes, 4x fewer DMAs);
   partials are summed in fp32 host-side.
"""

import math

import ml_dtypes
import numpy as np

S = 2048
D = 2048
HD = 128
NH = 16
N_CORES = 8
SQ = 512          # free-dim chunk for matmuls / psum tiles
NJ = S // SQ      # 4 s-chunks
KP = D // 256     # 8 contraction k-pair chunks (DoubleRow: 256 each)
NST = S // 128    # 16 s-tiles of 128
BF16 = ml_dtypes.bfloat16
F16 = np.float16
F8 = ml_dtypes.float8_e4m3

_MODULE_CACHE = {}


def _build_module():
    from contextlib import ExitStack

    import concourse.bass as bass
    import concourse.bacc as bacc
    import concourse.mybir as mybir
    import concourse.tile as tile

    f32 = mybir.dt.float32
    f16 = mybir.dt.float16
    bf16 = mybir.dt.bfloat16
    fp8 = mybir.dt.float8e4
    DR = mybir.MatmulPerfMode.DoubleRow
    ts = bass.ts

    nc = bacc.Bacc("TRN2", target_bir_lowering=False, debug=False,
                   num_devices=N_CORES)

    # DRAM I/O (identical program on all cores; per-core data differs)
    xh_d = nc.dram_tensor("xh", [1024, 2 * S], fp8, kind="ExternalInput").ap()
    xl_d = nc.dram_tensor("xl", [1024, 2 * S], fp8, kind="ExternalInput").ap()
    wh_d = nc.dram_tensor("wh", [1024, 2 * 1536], fp8,
                          kind="ExternalInput").ap()
    wl_d = nc.dram_tensor("wl", [1024, 2 * 1536], fp8,
                          kind="ExternalInput").ap()
    bias_d = nc.dram_tensor("bias", [12, 256], fp8, kind="ExternalInput").ap()
    woh_d = nc.dram_tensor("woh", [256, 2 * D], fp8, kind="ExternalInput").ap()
    wol_d = nc.dram_tensor("wol", [256, 2 * D], fp8, kind="ExternalInput").ap()
    stab_d = nc.dram_tensor("stab", [128, S], bf16, kind="ExternalInput").ap()
    ctab_d = nc.dram_tensor("ctab", [128, S], bf16, kind="ExternalInput").ap()
    cmask_d = nc.dram_tensor("cmask", [128, 128], bf16, kind="ExternalInput").ap()
    ident_d = nc.dram_tensor("ident", [128, 128], bf16, kind="ExternalInput").ap()
    out_d = nc.dram_tensor("out", [S, D], f16, kind="ExternalOutput").ap()

    inv_sqrt_hd = 1.0 / math.sqrt(HD)

    with tile.TileContext(nc) as tc, ExitStack() as ctx:
        # Long-lived pools first; x/w/tab/vt live in an inner scope that is
        # closed after the last QKV pass so the wo pool can reuse the space
        # (SBUF pool allocation is a LIFO stack).
        ps = ctx.enter_context(
            tc.tile_pool(name="ps", bufs=8, space=bass.MemorySpace.PSUM))
        const_p = ctx.enter_context(tc.tile_pool(name="const", bufs=1))
        qk_p = ctx.enter_context(tc.tile_pool(name="qk", bufs=4))
        v_p = ctx.enter_context(tc.tile_pool(name="v", bufs=2))
        scr_p = ctx.enter_context(tc.tile_pool(name="scr", bufs=10))
        p_p = ctx.enter_context(tc.tile_pool(name="p", bufs=6))
        attn_p = ctx.enter_context(tc.tile_pool(name="attn", bufs=4))
        af_p = ctx.enter_context(tc.tile_pool(name="af", bufs=3))
        bc_p = ctx.enter_context(tc.tile_pool(name="bc", bufs=2))
        rcp_p = ctx.enter_context(tc.tile_pool(name="rcp", bufs=2))
        out_p = ctx.enter_context(tc.tile_pool(name="outp", bufs=3))
        wo_p = ctx.enter_context(tc.tile_pool(name="wo", bufs=4))
        ctx2 = ctx.enter_context(ExitStack())
        x_p = ctx2.enter_context(tc.tile_pool(name="xp", bufs=16))
        w_p = ctx2.enter_context(tc.tile_pool(name="wp", bufs=16))
        tab_p = ctx2.enter_context(tc.tile_pool(name="tab", bufs=2))
        vt_p = ctx2.enter_context(tc.tile_pool(name="vt", bufs=2))

        # x^T (hi+lo fp8, k-pair layout) resident in SBUF; pair-0 weights
        # interleaved so the first QKV pass starts immediately and the
        # stream stays just ahead of PE consumption.
        xh_t, xl_t = [], []
        wh0, wl0 = [], []
        for kp in range(KP):
            t = x_p.tile([128, 2, S], fp8, tag="x", name="xh")
            nc.sync.dma_start(out=t[:], in_=xh_d[kp * 128:(kp + 1) * 128, :])
            xh_t.append(t)
            t = w_p.tile([128, 2, 768], fp8, tag="w", name="wh0")
            nc.sync.dma_start(out=t[:], in_=wh_d[kp * 128:(kp + 1) * 128,
                                                 0:1536])
            wh0.append(t)
            t = x_p.tile([128, 2, S], fp8, tag="x", name="xl")
            nc.sync.dma_start(out=t[:], in_=xl_d[kp * 128:(kp + 1) * 128, :])
            xl_t.append(t)
            t = w_p.tile([128, 2, 768], fp8, tag="w", name="wl0")
            nc.sync.dma_start(out=t[:], in_=wl_d[kp * 128:(kp + 1) * 128,
                                                 0:1536])
            wl0.append(t)

        # qkv bias first (pass jg=0's stop matmuls need it earliest),
        # then rope tables, then attention constants
        bias_t = []
        for i in range(12):
            t = const_p.tile([1, 2, 128], fp8, tag=f"b{i}")
            nc.sync.dma_start(out=t[:], in_=bias_d[i:i + 1, :])
            bias_t.append(t)
        stab = tab_p.tile([128, S], bf16, tag="tab")
        ctab = tab_p.tile([128, S], bf16, tag="tab")
        # first-half columns land first: rope j=0,1 unblocks ~1.6us earlier
        nc.sync.dma_start(out=stab[:, 0:1024], in_=stab_d[:, 0:1024])
        nc.sync.dma_start(out=ctab[:, 0:1024], in_=ctab_d[:, 0:1024])
        nc.sync.dma_start(out=stab[:, 1024:S], in_=stab_d[:, 1024:S])
        nc.sync.dma_start(out=ctab[:, 1024:S], in_=ctab_d[:, 1024:S])
        cmask = const_p.tile([128, 128], bf16, tag="c0")
        nc.sync.dma_start(out=cmask[:], in_=cmask_d[:])
        ident = const_p.tile([128, 128], bf16, tag="c1")
        nc.sync.dma_start(out=ident[:], in_=ident_d[:])
        ones2 = const_p.tile([1, 2, SQ], fp8, tag="c4")
        nc.vector.memset(ones2[:], 1.0)
        ones_col = const_p.tile([128, 1], bf16, tag="c5")
        nc.vector.memset(ones_col[:], 1.0)
        ones_f16 = const_p.tile([1, 128], f16, tag="c6")
        nc.vector.memset(ones_f16[:], 1.0)

        def load_pair_w(pair):
            whs, wls = [], []
            for kp in range(KP):
                t = w_p.tile([128, 2, 768], fp8, tag="w", name="wh1")
                nc.sync.dma_start(
                    out=t[:], in_=wh_d[kp * 128:(kp + 1) * 128,
                                       pair * 1536:(pair + 1) * 1536])
                whs.append(t)
                t = w_p.tile([128, 2, 768], fp8, tag="w", name="wl1")
                nc.sync.dma_start(
                    out=t[:], in_=wl_d[kp * 128:(kp + 1) * 128,
                                       pair * 1536:(pair + 1) * 1536])
                wls.append(t)
            return whs, wls

        def qkv_pass(w_tiles, pair, jms):
            """Accumulate 64*(x @ W + b) for the given (j, mat) pairs via
            3 fp8 DoubleRow chains.  jms must be 8 accs forming 4 rope/V
            pairs (consecutive); 3 pairs land in 2-bank psb tiles, the 4th
            in two pss tiles.  Returns {(j, m): psum AP [128, SQ] f32}."""
            whs, wls = w_tiles
            accs = {}
            for jm in jms:
                accs[jm] = ps.tile([128, SQ], f32, tag="ps",
                                   name="qkv_acc")[:]
            for kp in range(KP):
                for ci, (cx, cw) in enumerate(
                        [(xh_t, whs), (xl_t, whs), (xh_t, wls)]):
                    for (j, m) in jms:
                        nc.tensor.matmul(
                            accs[(j, m)],
                            cw[kp][:, :, ts(m, 128)],
                            cx[kp][:, :, ts(j, SQ)],
                            start=(kp == 0 and ci == 0), stop=False,
                            perf_mode=DR)
            for (j, m) in jms:
                nc.tensor.matmul(
                    accs[(j, m)],
                    bias_t[pair * 6 + m][:, :, :],
                    ones2[:, :, :],
                    start=False, stop=True, perf_mode=DR)
            return accs

        def rope(j, A, B, dsts):
            """A=[lo ha|lo hb], B=[hi ha|hi hb] pair-interleaved psum tiles
            (64x scale); writes per-head contiguous rotated [128, SQ] slices
            into dsts[0] (head a) and dsts[1] (head b).  Tables carry the
            1/64:  rot_lo = lo*sin - hi*cos ; rot_hi = hi*sin + lo*cos."""
            sl = stab[:, ts(j, SQ)]
            cl = ctab[:, ts(j, SQ)]
            # drain psum -> bf16 SBUF on the scalar engine first: frees the
            # psum bank after 2 ops (not 4) and makes every DVE op below
            # all-bf16/SBUF -> 2x DVE rate
            A2 = scr_p.tile([128, SQ], bf16, tag="scr")
            nc.scalar.copy(A2[:], A)
            B2 = scr_p.tile([128, SQ], bf16, tag="scr")
            nc.scalar.copy(B2[:], B)
            t1 = scr_p.tile([128, SQ], bf16, tag="scr")
            nc.vector.tensor_mul(t1[:], A2[:], sl)
            t2 = scr_p.tile([128, SQ], bf16, tag="scr")
            nc.vector.tensor_mul(t2[:], B2[:], cl)
            t3 = scr_p.tile([128, SQ], bf16, tag="scr")
            nc.vector.tensor_mul(t3[:], B2[:], sl)
            t4 = scr_p.tile([128, SQ], bf16, tag="scr")
            nc.vector.tensor_mul(t4[:], A2[:], cl)
            for hh in range(2):
                hs = slice(64 * hh, 64 * hh + 64)
                nc.vector.tensor_sub(dsts[hh][0:64, ts(j, SQ)],
                                     t1[hs, :], t2[hs, :])
                nc.vector.tensor_add(dsts[hh][64:128, ts(j, SQ)],
                                     t3[hs, :], t4[hs, :])

        # attn pair tiles for o_proj fp8 chains: [128, 2, S], slot = head
        a2h = [attn_p.tile([128, 2, S], fp8, tag="attn", name="a2h")
               for _ in range(2)]
        a2l = [attn_p.tile([128, 2, S], fp8, tag="attn", name="a2l")
               for _ in range(2)]

        def attn_j(pair, j, qT, kT, vs):
            """Emit attention for s-chunk j of the given head pair."""
            ndiag = 4 * j + 4
            apsum = dpsum = None
            for i in range(ndiag):
                r = i - 4 * j
                # columns left of a diagonal tile's valid triangle are
                # fully masked -> skip them in QK/exp/PV/denom entirely
                off = 128 * r if r > 0 else 0
                lg = [ps.tile([128, SQ], f32, tag="ps", name="lg")
                      for _ in range(2)]
                # logits^T[sk,sq] = K^T.T @ Q^T; on diagonal tiles the
                # causal -9e15 triangle is accumulated on the PE itself
                # (cmask^T @ ident, 128 cols) - no DVE hop before exp
                for hh in range(2):
                    nc.tensor.matmul(lg[hh][:, off:SQ],
                                     kT[hh][:, ts(i, 128)],
                                     qT[hh][:, j * SQ + off:(j + 1) * SQ],
                                     start=True, stop=(r < 0))
                    if r >= 0:
                        nc.tensor.matmul(lg[hh][:, off:off + 128],
                                         cmask[:], ident[:],
                                         start=False, stop=True)
                if i == 0:
                    # allocated after the first logits tiles so the next
                    # chunk's QK matmuls can start while the previous
                    # chunk is still normalizing
                    apsum = [ps.tile([128, SQ], f32, tag="ps",
                                     name="apsum")[:] for _ in range(2)]
                    dpsum = ps.tile([33, SQ], f32, tag="ps",
                                    name="dpsum")
                last = (i == ndiag - 1)
                for hh in range(2):
                    p_t = p_p.tile([128, SQ], bf16, tag="p")
                    nc.scalar.activation(
                        p_t[:, off:SQ], lg[hh][:, off:SQ],
                        mybir.ActivationFunctionType.Exp,
                        scale=inv_sqrt_hd)
                    nc.tensor.matmul(dpsum[32 * hh:32 * hh + 1, off:SQ],
                                     ones_col[:, 0:1],
                                     p_t[:, off:SQ],
                                     start=(i == 0), stop=last)
                    nc.tensor.matmul(apsum[hh][:, off:SQ],
                                     vs[hh][:, i, :],
                                     p_t[:, off:SQ],
                                     start=(i == 0), stop=last)
            # tail emitted stage-by-stage across both heads so the DVE
            # stream has no head-of-line blocking (recips don't trap the
            # bc copies behind the PE broadcast matmuls)
            rcs, bcps, bcs = [], [], []
            for hh in range(2):
                # fp16 reciprocal: full-rate matmul dtype, 2^-11 relative
                # precision is ample for softmax denominators
                rc = rcp_p.tile([1, SQ], f16, tag="rcp")
                with nc.allow_low_precision(reason="fp16 1/denom"):
                    nc.vector.reciprocal(rc[:],
                                         dpsum[32 * hh:32 * hh + 1, :])
                rcs.append(rc)
            for hh in range(2):
                # broadcast 1/denom across partitions via a K=1 matmul
                bcp = ps.tile([128, SQ], f32, tag="ps", name="bcp")
                nc.tensor.matmul(bcp[:], ones_f16[0:1, :], rcs[hh][0:1, :],
                                 start=True, stop=True)
                bcps.append(bcp)
            for hh in range(2):
                bc = bc_p.tile([128, SQ], f32, tag="bc")
                nc.vector.tensor_copy(bc[:], bcps[hh][:])
                bcs.append(bc)
            for hh in range(2):
                # normalized attn^T -> f16, then fp8 hi/lo pair layout
                af = af_p.tile([128, SQ], f16, tag="af")
                nc.vector.tensor_mul(af[:], apsum[hh], bcs[hh][:])
                nc.gpsimd.tensor_copy(a2h[pair][:, hh, ts(j, SQ)], af[:])
                nc.gpsimd.tensor_sub(a2l[pair][:, hh, ts(j, SQ)],
                                     af[:], a2h[pair][:, hh, ts(j, SQ)])

        ncopy = [0]
        wo2 = []

        def oproj_block(st):
            """o_proj partial for s-tile st: out[s,:] = sum_h attn_h @ wo_h
            (fp8 chains over both head pairs); one wide row DMA per tile
            (narrow DMAs for the last tile - shorter shutdown chain)."""
            narrow = (st == NST - 1)
            orow = out_p.tile([128, D], f16, tag="outp")
            for eg in range(2):
                ops = [ps.tile([128, SQ], f32, tag="ps", name="oproj")[:]
                       for _ in range(2)]
                for g in range(2):
                    woh_t, wol_t = wo2[g]
                    for ci, (ca, cw) in enumerate(
                            [(a2h[g], woh_t), (a2l[g], woh_t),
                             (a2h[g], wol_t)]):
                        for ei in range(2):
                            e = 2 * eg + ei
                            nc.tensor.matmul(
                                ops[ei],
                                ca[:, :, ts(st, 128)],
                                cw[:, :, ts(e, SQ)],
                                start=(g == 0 and ci == 0),
                                stop=(g == 1 and ci == 2),
                                perf_mode=DR)
                for ei in range(2):
                    e = 2 * eg + ei
                    if ncopy[0] % 2 == 0:
                        nc.scalar.mul(orow[:, ts(e, SQ)], ops[ei],
                                      1.0 / 64.0)
                    else:
                        nc.vector.tensor_scalar_mul(orow[:, ts(e, SQ)],
                                                    ops[ei], 1.0 / 64.0)
                    ncopy[0] += 1
                    if narrow:
                        nc.sync.dma_start(
                            out=out_d[st * 128:(st + 1) * 128,
                                      e * SQ:(e + 1) * SQ],
                            in_=orow[:, ts(e, SQ)])
            if not narrow:
                nc.sync.dma_start(out=out_d[st * 128:(st + 1) * 128, :],
                                  in_=orow[:])

        w1_tiles = [None]

        for pair in range(2):
            w_tiles = (wh0, wl0) if pair == 0 else w1_tiles[0]

            # Q/K projection (+bias) and rope -> per-head contiguous tiles
            qT = [qk_p.tile([128, S], bf16, tag="qk", name="qT")
                  for _ in range(2)]
            kT = [qk_p.tile([128, S], bf16, tag="qk", name="kT")
                  for _ in range(2)]
            for jg in range(2):
                accs = qkv_pass(w_tiles, pair,
                                [(j, m) for j in (2 * jg, 2 * jg + 1)
                                 for m in range(4)])
                for j in (2 * jg, 2 * jg + 1):
                    rope(j, accs[(j, 0)], accs[(j, 1)], qT)
                    rope(j, accs[(j, 2)], accs[(j, 3)], kT)

            # V projection -> V^T -> transpose to [s, d] layout per head.
            # Two 4-acc passes: pass A's psum banks drain (copies + DMA-xbar
            # transposes) while pass B computes, so attention starts with
            # banks already free.
            vts = [vt_p.tile([128, S], bf16, tag="vt", name="vt0"),
                   vt_p.tile([128, S], bf16, tag="vt", name="vt1")]
            vs = [v_p.tile([128, NST, 128], bf16, tag="v", name="v0"),
                  v_p.tile([128, NST, 128], bf16, tag="v", name="v1")]
            for jg in range(2):
                vacc = qkv_pass(w_tiles, pair,
                                [(j, 4 + hh) for j in (2 * jg, 2 * jg + 1)
                                 for hh in range(2)])
                for ji, j in enumerate((2 * jg, 2 * jg + 1)):
                    for hh in range(2):
                        # alternate engines so the psum banks drain fast
                        if (ji + hh) % 2 == 0:
                            nc.scalar.mul(vts[hh][:, ts(j, SQ)],
                                          vacc[(j, 4 + hh)], 1.0 / 64.0)
                        else:
                            nc.vector.tensor_scalar_mul(
                                vts[hh][:, ts(j, SQ)],
                                vacc[(j, 4 + hh)], 1.0 / 64.0)
                        nc.sync.dma_start_transpose(
                            vs[hh][:, 4 * j:4 * j + 4, :],
                            vts[hh][:, ts(j, SQ)])

            if pair == 0:
                # pair-1 weights stream during pair-0 attention (DMA idle;
                # the w pool bufs free as pair-0''s last pass retires)
                w1_tiles[0] = load_pair_w(1)
            else:
                # wo streams during pair-1 attention
                for g in range(2):
                    th = wo_p.tile([128, 2, D], fp8, tag="wo", name="woh_t")
                    nc.sync.dma_start(out=th[:],
                                      in_=woh_d[g * 128:(g + 1) * 128, :])
                    tl = wo_p.tile([128, 2, D], fp8, tag="wo", name="wol_t")
                    nc.sync.dma_start(out=tl[:],
                                      in_=wol_d[g * 128:(g + 1) * 128, :])
                    wo2.append((th, tl))

            # attention for the pair's two heads, processed jointly.  For
            # pair 1, o_proj s-tile blocks are interleaved at j boundaries:
            # o_proj rows 512j..512j+511 need only chunks <= j of both pairs,
            # and the extra PE work fills the softmax-tail bubbles.
            for j in range(NJ):
                attn_j(pair, j, qT, kT, vs)

        for st in range(NST):
            oproj_block(st)

    nc.compile()
    return nc


def _split8(a):
    hi = np.clip(a, -240.0, 240.0).astype(F8)
    lo = (a - hi.astype(np.float32)).astype(F8)
    return hi, lo


def _pairify(a):
    """[K, C] -> [K/2, 2C]: row kp*128+p holds k-chunks (2kp, 2kp+1) side
    by side (DoubleRow k-pair layout)."""
    Kd, C = a.shape
    return np.ascontiguousarray(
        a.reshape(Kd // 256, 2, 128, C).transpose(0, 2, 1, 3)
        .reshape(Kd // 2, 2 * C))


def _host_inputs(x, w_qkv, b_qkv, w_o):
    """Build the 8 per-core input maps."""
    x = np.asarray(x, dtype=np.float32)
    w_qkv = np.asarray(w_qkv, dtype=np.float32)
    b_qkv = np.asarray(b_qkv, dtype=np.float32)
    w_o = np.asarray(w_o, dtype=np.float32)

    # rope tables (reference swaps sin/cos roles; we follow the math:
    # q_rot = q*sin(emb) + rotate_half(q)*cos(emb)); 1/64 de-scales the
    # 64x weight scaling used to keep fp8 weight parts in normal range
    inv_freq = 1.0 / (10000.0 ** (np.arange(0, HD, 2, dtype=np.float32) / HD))
    t = np.arange(S, dtype=np.float32)
    freq = np.einsum("s,f->sf", t, inv_freq)          # [S, 64]
    sinT = np.sin(freq).T.astype(np.float32) / 64.0   # [64, S]
    cosT = np.cos(freq).T.astype(np.float32) / 64.0
    stab = np.concatenate([sinT, sinT], 0).astype(BF16)   # [128, S]
    ctab = np.concatenate([cosT, cosT], 0).astype(BF16)

    p_idx = np.arange(128)[:, None]
    f_idx = np.arange(128)[None, :]
    # used as matmul lhsT (accumulated as cmask.T @ ident into the logits):
    # effective additive mask[sk, sq] = cmask[sq, sk] = -9e15 where sq < sk
    cmask = np.where(p_idx >= f_idx, 0.0, -9e15).astype(BF16)
    ident = np.eye(128, dtype=np.float32).astype(BF16)

    def head_w(h):
        base = h * 3 * HD
        return (w_qkv[:, base:base + HD],
                w_qkv[:, base + HD:base + 2 * HD],
                w_qkv[:, base + 2 * HD:base + 3 * HD])

    def head_b(h):
        base = h * 3 * HD
        return (b_qkv[base:base + HD],
                b_qkv[base + HD:base + 2 * HD],
                b_qkv[base + 2 * HD:base + 3 * HD])

    in_maps = []
    for c in range(N_CORES):
        b = c // 4
        heads = [4 * (c % 4) + i for i in range(4)]
        xT = np.ascontiguousarray(x[b].T)               # [D, S] f32
        xh, xl = _split8(xT)
        xh = _pairify(xh)
        xl = _pairify(xl)

        mats, bvec = [], []
        for pair in range(2):
            ha, hb = heads[2 * pair], heads[2 * pair + 1]
            wq_a, wk_a, wv_a = head_w(ha)
            wq_b, wk_b, wv_b = head_w(hb)
            bq_a, bk_a, bv_a = head_b(ha)
            bq_b, bk_b, bv_b = head_b(hb)
            mats += [
                np.concatenate([wq_a[:, :64], wq_b[:, :64]], 1),
                np.concatenate([wq_a[:, 64:], wq_b[:, 64:]], 1),
                np.concatenate([wk_a[:, :64], wk_b[:, :64]], 1),
                np.concatenate([wk_a[:, 64:], wk_b[:, 64:]], 1),
                wv_a, wv_b,
            ]
            bvec += [
                np.concatenate([bq_a[:64], bq_b[:64]]),
                np.concatenate([bq_a[64:], bq_b[64:]]),
                np.concatenate([bk_a[:64], bk_b[:64]]),
                np.concatenate([bk_a[64:], bk_b[64:]]),
                bv_a, bv_b,
            ]
        w_all = np.concatenate(mats, 1) * 64.0               # [D, 1536]

        def _w_layout(a):
            # [2048, 1536] -> rows kp*128+p, cols pair*1536 + slot*768 + c
            return np.ascontiguousarray(
                a.reshape(KP, 2, 128, 2, 768).transpose(0, 2, 3, 1, 4)
                .reshape(1024, 3072))

        wh8, wl8 = _split8(w_all)
        wh8 = _w_layout(wh8)
        wl8 = _w_layout(wl8)
        bias_rows = np.zeros((12, 256), dtype=F8)
        for i, bv in enumerate(bvec):
            bh, bl = _split8(bv * 64.0)
            bias_rows[i, :128] = bh
            bias_rows[i, 128:] = bl

        wo_all = np.concatenate(
            [w_o[h * HD:(h + 1) * HD, :] for h in heads], 0) * 64.0  # [512,D]
        woh8, wol8 = _split8(wo_all)
        # [512, D] -> rows g*128+p, cols slot*D+e (slot = head-in-pair)
        woh8 = np.ascontiguousarray(
            woh8.reshape(2, 2, 128, D).transpose(0, 2, 1, 3).reshape(256, 2 * D))
        wol8 = np.ascontiguousarray(
            wol8.reshape(2, 2, 128, D).transpose(0, 2, 1, 3).reshape(256, 2 * D))

        in_maps.append({
            "xh": xh, "xl": xl, "wh": wh8, "wl": wl8, "bias": bias_rows,
            "woh": woh8, "wol": wol8,
            "stab": stab, "ctab": ctab, "cmask": cmask, "ident": ident,
        })
    return in_maps


def _run(in_maps, trace=False):
    from concourse.bass_utils import run_bass_kernel_spmd
    if "nc" not in _MODULE_CACHE:
        _MODULE_CACHE["nc"] = _build_module()
    nc = _MODULE_CACHE["nc"]
    return run_bass_kernel_spmd(nc, in_maps, core_ids=list(range(N_CORES)),
                                trace=trace)


def kernel(x, w_qkv, b_qkv, w_o, b_o, _trace=False, _return_res=False):
    in_maps = _host_inputs(x, w_qkv, b_qkv, w_o)
    res = _run(in_maps, trace=_trace)
    out = np.zeros((2, S, D), dtype=np.float32)
    for c in range(N_CORES):
        out[c // 4] += res.results[c]["out"].astype(np.float32)
    out += np.asarray(b_o, dtype=np.float32)[None, None, :]
    if _return_res:
        return out, res
    return out


# revision 63
# speedup vs baseline: 1.0378x; 1.0378x over previous
"""Trainium2 Bass kernel for fused multi-head causal attention.

Module: out = o_proj(causal_attention(rope_swapped(qkv_proj(x)))).
Shapes: x [2, 2048, 2048], 16 heads, head_dim 128.

Sharding (8 cores): batch (2) x head-group (4 groups of 4 heads).
Each core computes qkv projection + attention for its 4 heads of its
batch, then a partial o_proj against its slice of w_o rows.  The
all-reduce after o_proj is done host-side by summing the 4 partials
per batch (mathematically identical, avoids device collectives).

Device-kernel design notes:
 - QKV and o_proj matmuls run in fp8-e4m3 DoubleRow perf mode (0.5
   cycles/row on PE) with hi+lo error compensation: a ~= hi(a) + lo(a)
   with both parts fp8; a@b ~= ah@bh + al@bh + ah@bl (lo x lo dropped,
   ~0.1% error).  3 chains x half-rate = 0.75x the bf16 PE cost.
   Weights are scaled by 64 host-side so their fp8 parts stay normal;
   the 1/64 is folded into the rope tables / V copy / output copy.
 - DoubleRow contracts 2 k-chunks of 128 per instruction; operands are
   laid out [128 partitions, 2, free] with the paired k-chunks adjacent
   in the free dim (prepared host-side).
 - The qkv bias is added with a single fp8 DoubleRow rank-1 matmul per
   output tile: lhsT [1, 2, 128] = (bias_hi | bias_lo), rhs = ones.
 - QKV projection computes Q^T/K^T/V^T ([head_dim, S] layout) directly:
   out = W_slice.T @ x^T, so attention's QK^T matmul needs no transposes.
 - Q/K head pairs are interleaved ([q_ha_lo | q_hb_lo] on 128 partitions)
   so RoPE's rotate_half partner lives at the SAME partition of a sibling
   tile -> full-width DVE ops (3 ops/element, no cross-partition shuffle).
 - Scores are computed transposed ([sk, sq]) so the PV matmul consumes
   exp(scores) directly; softmax denominators come from a ones-vector
   matmul; normalization is folded into the PSUM->SBUF copy of attn^T,
   which also emits the fp8 hi/lo pair layout for the o_proj chains.
 - Causal masking: fully-masked columns of diagonal score tiles are
   skipped outright; the remaining width-128 triangle is accumulated
   into the logits psum ON THE PE (cmask^T @ ident) so no vector-engine
   hop sits between QK^T and the exp.
 - V^T -> V transposes go through the DMA xbar (dma_start_transpose),
   off the PE/psum entirely; V projection runs as two 4-acc passes so
   its psum banks drain while the second pass computes.
 - Attention core (QK^T, exp, PV, denominators) stays bf16: a single
   uncompensated fp8 tensor there would inject ~3.6% output error
   (the hi/lo compensation only pays at contraction >= 256).
 - Output rows are assembled [128, 2048] f16 in SBUF and written with
   one wide DMA per s-tile (halves output bytes, 4x fewer DMAs);
   partials are summed in fp32 host-side.
"""

import math

import ml_dtypes
import numpy as np

S = 2048
D = 2048
HD = 128
NH = 16
N_CORES = 8
SQ = 512          # free-dim chunk for matmuls / psum tiles
NJ = S // SQ      # 4 s-chunks
KP = D // 256     # 8 contraction k-pair chunks (DoubleRow: 256 each)
NST = S // 128    # 16 s-tiles of 128
BF16 = ml_dtypes.bfloat16
F16 = np.float16
F8 = ml_dtypes.float8_e4m3

_MODULE_CACHE = {}


def _build_module():
    from contextlib import ExitStack

    import concourse.bass as bass
    import concourse.bacc as bacc
    import concourse.mybir as mybir
    import concourse.tile as tile

    f32 = mybir.dt.float32
    f16 = mybir.dt.float16
    bf16 = mybir.dt.bfloat16
    fp8 = mybir.dt.float8e4
    DR = mybir.MatmulPerfMode.DoubleRow
    ts = bass.ts

    nc = bacc.Bacc("TRN2", target_bir_lowering=False, debug=False,
                   num_devices=N_CORES)

    # DRAM I/O (identical program on all cores; per-core data differs)
    xh_d = nc.dram_tensor("xh", [1024, 2 * S], fp8, kind="ExternalInput").ap()
    xl_d = nc.dram_tensor("xl", [1024, 2 * S], fp8, kind="ExternalInput").ap()
    wh_d = nc.dram_tensor("wh", [1024, 2 * 1536], fp8,
                          kind="ExternalInput").ap()
    wl_d = nc.dram_tensor("wl", [1024, 2 * 1536], fp8,
                          kind="ExternalInput").ap()
    bias_d = nc.dram_tensor("bias", [12, 256], fp8, kind="ExternalInput").ap()
    woh_d = nc.dram_tensor("woh", [256, 2 * D], fp8, kind="ExternalInput").ap()
    wol_d = nc.dram_tensor("wol", [256, 2 * D], fp8, kind="ExternalInput").ap()
    stab_d = nc.dram_tensor("stab", [128, S], bf16, kind="ExternalInput").ap()
    ctab_d = nc.dram_tensor("ctab", [128, S], bf16, kind="ExternalInput").ap()
    cmask_d = nc.dram_tensor("cmask", [128, 128], bf16, kind="ExternalInput").ap()
    ident_d = nc.dram_tensor("ident", [128, 128], bf16, kind="ExternalInput").ap()
    out_d = nc.dram_tensor("out", [S, D], f16, kind="ExternalOutput").ap()

    inv_sqrt_hd = 1.0 / math.sqrt(HD)

    with tile.TileContext(nc) as tc, ExitStack() as ctx:
        # Long-lived pools first; x/w/tab/vt live in an inner scope that is
        # closed after the last QKV pass so the wo pool can reuse the space
        # (SBUF pool allocation is a LIFO stack).
        ps = ctx.enter_context(
            tc.tile_pool(name="ps", bufs=8, space=bass.MemorySpace.PSUM))
        const_p = ctx.enter_context(tc.tile_pool(name="const", bufs=1))
        qk_p = ctx.enter_context(tc.tile_pool(name="qk", bufs=4))
        v_p = ctx.enter_context(tc.tile_pool(name="v", bufs=2))
        scr_p = ctx.enter_context(tc.tile_pool(name="scr", bufs=10))
        p_p = ctx.enter_context(tc.tile_pool(name="p", bufs=6))
        attn_p = ctx.enter_context(tc.tile_pool(name="attn", bufs=4))
        af_p = ctx.enter_context(tc.tile_pool(name="af", bufs=3))
        bc_p = ctx.enter_context(tc.tile_pool(name="bc", bufs=2))
        rcp_p = ctx.enter_context(tc.tile_pool(name="rcp", bufs=2))
        out_p = ctx.enter_context(tc.tile_pool(name="outp", bufs=3))
        wo_p = ctx.enter_context(tc.tile_pool(name="wo", bufs=4))
        ctx2 = ctx.enter_context(ExitStack())
        x_p = ctx2.enter_context(tc.tile_pool(name="xp", bufs=16))
        w_p = ctx2.enter_context(tc.tile_pool(name="wp", bufs=16))
        tab_p = ctx2.enter_context(tc.tile_pool(name="tab", bufs=2))
        vt_p = ctx2.enter_context(tc.tile_pool(name="vt", bufs=2))

        # x^T (hi+lo fp8, k-pair layout) resident in SBUF; pair-0 weights
        # interleaved so the first QKV pass starts immediately and the
        # stream stays just ahead of PE consumption.
        xh_t, xl_t = [], []
        wh0, wl0 = [], []
        for kp in range(KP):
            t = x_p.tile([128, 2, S], fp8, tag="x", name="xh")
            nc.sync.dma_start(out=t[:], in_=xh_d[kp * 128:(kp + 1) * 128, :])
            xh_t.append(t)
            t = w_p.tile([128, 2, 768], fp8, tag="w", name="wh0")
            nc.sync.dma_start(out=t[:], in_=wh_d[kp * 128:(kp + 1) * 128,
                                                 0:1536])
            wh0.append(t)
            t = x_p.tile([128, 2, S], fp8, tag="x", name="xl")
            nc.sync.dma_start(out=t[:], in_=xl_d[kp * 128:(kp + 1) * 128, :])
            xl_t.append(t)
            t = w_p.tile([128, 2, 768], fp8, tag="w", name="wl0")
            nc.sync.dma_start(out=t[:], in_=wl_d[kp * 128:(kp + 1) * 128,
                                                 0:1536])
            wl0.append(t)

        # qkv bias first (pass jg=0's stop matmuls need it earliest),
        # then rope tables, then attention constants
        bias_t = []
        for i in range(12):
            t = const_p.tile([1, 2, 128], fp8, tag=f"b{i}")
            nc.sync.dma_start(out=t[:], in_=bias_d[i:i + 1, :])
            bias_t.append(t)
        stab = tab_p.tile([128, S], bf16, tag="tab")
        ctab = tab_p.tile([128, S], bf16, tag="tab")
        # first-half columns land first: rope j=0,1 unblocks ~1.6us earlier
        nc.sync.dma_start(out=stab[:, 0:1024], in_=stab_d[:, 0:1024])
        nc.sync.dma_start(out=ctab[:, 0:1024], in_=ctab_d[:, 0:1024])
        nc.sync.dma_start(out=stab[:, 1024:S], in_=stab_d[:, 1024:S])
        nc.sync.dma_start(out=ctab[:, 1024:S], in_=ctab_d[:, 1024:S])
        cmask = const_p.tile([128, 128], bf16, tag="c0")
        nc.sync.dma_start(out=cmask[:], in_=cmask_d[:])
        ident = const_p.tile([128, 128], bf16, tag="c1")
        nc.sync.dma_start(out=ident[:], in_=ident_d[:])
        ones2 = const_p.tile([1, 2, SQ], fp8, tag="c4")
        nc.vector.memset(ones2[:], 1.0)
        ones_col = const_p.tile([128, 1], bf16, tag="c5")
        nc.vector.memset(ones_col[:], 1.0)
        ones_f16 = const_p.tile([1, 128], f16, tag="c6")
        nc.vector.memset(ones_f16[:], 1.0)

        def load_pair_w(pair):
            whs, wls = [], []
            for kp in range(KP):
                t = w_p.tile([128, 2, 768], fp8, tag="w", name="wh1")
                nc.sync.dma_start(
                    out=t[:], in_=wh_d[kp * 128:(kp + 1) * 128,
                                       pair * 1536:(pair + 1) * 1536])
                whs.append(t)
                t = w_p.tile([128, 2, 768], fp8, tag="w", name="wl1")
                nc.sync.dma_start(
                    out=t[:], in_=wl_d[kp * 128:(kp + 1) * 128,
                                       pair * 1536:(pair + 1) * 1536])
                wls.append(t)
            return whs, wls

        def qkv_pass(w_tiles, pair, jms):
            """Accumulate 64*(x @ W + b) for the given (j, mat) pairs via
            3 fp8 DoubleRow chains.  jms must be 8 accs forming 4 rope/V
            pairs (consecutive); 3 pairs land in 2-bank psb tiles, the 4th
            in two pss tiles.  Returns {(j, m): psum AP [128, SQ] f32}."""
            whs, wls = w_tiles
            accs = {}
            for jm in jms:
                accs[jm] = ps.tile([128, SQ], f32, tag="ps",
                                   name="qkv_acc")[:]
            for kp in range(KP):
                for ci, (cx, cw) in enumerate(
                        [(xh_t, whs), (xl_t, whs), (xh_t, wls)]):
                    for (j, m) in jms:
                        nc.tensor.matmul(
                            accs[(j, m)],
                            cw[kp][:, :, ts(m, 128)],
                            cx[kp][:, :, ts(j, SQ)],
                            start=(kp == 0 and ci == 0), stop=False,
                            perf_mode=DR)
            for (j, m) in jms:
                nc.tensor.matmul(
                    accs[(j, m)],
                    bias_t[pair * 6 + m][:, :, :],
                    ones2[:, :, :],
                    start=False, stop=True, perf_mode=DR)
            return accs

        def rope(j, A, B, dsts):
            """A=[lo ha|lo hb], B=[hi ha|hi hb] pair-interleaved psum tiles
            (64x scale); writes per-head contiguous rotated [128, SQ] slices
            into dsts[0] (head a) and dsts[1] (head b).  Tables carry the
            1/64:  rot_lo = lo*sin - hi*cos ; rot_hi = hi*sin + lo*cos."""
            sl = stab[:, ts(j, SQ)]
            cl = ctab[:, ts(j, SQ)]
            # drain psum -> bf16 SBUF on the scalar engine first: frees the
            # psum bank after 2 ops (not 4) and makes every DVE op below
            # all-bf16/SBUF -> 2x DVE rate
            A2 = scr_p.tile([128, SQ], bf16, tag="scr")
            nc.scalar.copy(A2[:], A)
            B2 = scr_p.tile([128, SQ], bf16, tag="scr")
            nc.scalar.copy(B2[:], B)
            t1 = scr_p.tile([128, SQ], bf16, tag="scr")
            nc.vector.tensor_mul(t1[:], A2[:], sl)
            t2 = scr_p.tile([128, SQ], bf16, tag="scr")
            nc.vector.tensor_mul(t2[:], B2[:], cl)
            t3 = scr_p.tile([128, SQ], bf16, tag="scr")
            nc.vector.tensor_mul(t3[:], B2[:], sl)
            t4 = scr_p.tile([128, SQ], bf16, tag="scr")
            nc.vector.tensor_mul(t4[:], A2[:], cl)
            for hh in range(2):
                hs = slice(64 * hh, 64 * hh + 64)
                nc.vector.tensor_sub(dsts[hh][0:64, ts(j, SQ)],
                                     t1[hs, :], t2[hs, :])
                nc.vector.tensor_add(dsts[hh][64:128, ts(j, SQ)],
                                     t3[hs, :], t4[hs, :])

        # attn pair tiles for o_proj fp8 chains: [128, 2, S], slot = head
        a2h = [attn_p.tile([128, 2, S], fp8, tag="attn", name="a2h")
               for _ in range(2)]
        a2l = [attn_p.tile([128, 2, S], fp8, tag="attn", name="a2l")
               for _ in range(2)]

        def attn_j(pair, j, qT, kT, vs, pending=None):
            """Emit attention for s-chunk j of the given head pair.  The
            previous chunk's softmax tail (broadcast matmuls etc.) is
            emitted AFTER this chunk's first two i-steps so the in-order
            PE stream is never head-of-line blocked waiting on the
            reciprocal; returns this chunk's own tail closure."""
            ndiag = 4 * j + 4
            apsum = dpsum = None
            for i in range(ndiag):
                r = i - 4 * j
                # columns left of a diagonal tile's valid triangle are
                # fully masked -> skip them in QK/exp/PV/denom entirely
                off = 128 * r if r > 0 else 0
                lg = [ps.tile([128, SQ], f32, tag="ps", name="lg")
                      for _ in range(2)]
                # logits^T[sk,sq] = K^T.T @ Q^T; on diagonal tiles the
                # causal -9e15 triangle is accumulated on the PE itself
                # (cmask^T @ ident, 128 cols) - no DVE hop before exp
                for hh in range(2):
                    nc.tensor.matmul(lg[hh][:, off:SQ],
                                     kT[hh][:, ts(i, 128)],
                                     qT[hh][:, j * SQ + off:(j + 1) * SQ],
                                     start=True, stop=(r < 0))
                    if r >= 0:
                        nc.tensor.matmul(lg[hh][:, off:off + 128],
                                         cmask[:], ident[:],
                                         start=False, stop=True)
                if i == 0:
                    # allocated after the first logits tiles so the next
                    # chunk's QK matmuls can start while the previous
                    # chunk is still normalizing
                    apsum = [ps.tile([128, SQ], f32, tag="ps",
                                     name="apsum")[:] for _ in range(2)]
                    dpsum = ps.tile([33, SQ], f32, tag="ps",
                                    name="dpsum")
                last = (i == ndiag - 1)
                for hh in range(2):
                    p_t = p_p.tile([128, SQ], bf16, tag="p")
                    nc.scalar.activation(
                        p_t[:, off:SQ], lg[hh][:, off:SQ],
                        mybir.ActivationFunctionType.Exp,
                        scale=inv_sqrt_hd)
                    nc.tensor.matmul(dpsum[32 * hh:32 * hh + 1, off:SQ],
                                     ones_col[:, 0:1],
                                     p_t[:, off:SQ],
                                     start=(i == 0), stop=last)
                    nc.tensor.matmul(apsum[hh][:, off:SQ],
                                     vs[hh][:, i, :],
                                     p_t[:, off:SQ],
                                     start=(i == 0), stop=last)
                if i == 1 and pending is not None:
                    pending()
                    pending = None
            if pending is not None:
                pending()

            # recips issue now (DVE, frees dpsum); the PE-side broadcast
            # and the normalize run from the deferred closure
            rcs = []
            for hh in range(2):
                # fp16 reciprocal: full-rate matmul dtype, 2^-11 relative
                # precision is ample for softmax denominators
                rc = rcp_p.tile([1, SQ], f16, tag="rcp")
                with nc.allow_low_precision(reason="fp16 1/denom"):
                    nc.vector.reciprocal(rc[:],
                                         dpsum[32 * hh:32 * hh + 1, :])
                rcs.append(rc)
            my_apsum = apsum

            def tail():
                bcps, bcs = [], []
                for hh in range(2):
                    # broadcast 1/denom across partitions via a K=1 matmul
                    bcp = ps.tile([128, SQ], f32, tag="ps", name="bcp")
                    nc.tensor.matmul(bcp[:], ones_f16[0:1, :],
                                     rcs[hh][0:1, :],
                                     start=True, stop=True)
                    bcps.append(bcp)
                for hh in range(2):
                    bc = bc_p.tile([128, SQ], f32, tag="bc")
                    nc.vector.tensor_copy(bc[:], bcps[hh][:])
                    bcs.append(bc)
                for hh in range(2):
                    # normalized attn^T -> f16, then fp8 hi/lo pair layout
                    af = af_p.tile([128, SQ], f16, tag="af")
                    nc.vector.tensor_mul(af[:], my_apsum[hh], bcs[hh][:])
                    nc.gpsimd.tensor_copy(a2h[pair][:, hh, ts(j, SQ)],
                                          af[:])
                    nc.gpsimd.tensor_sub(a2l[pair][:, hh, ts(j, SQ)],
                                         af[:],
                                         a2h[pair][:, hh, ts(j, SQ)])
            return tail

        ncopy = [0]
        wo2 = []

        def oproj_block(st):
            """o_proj partial for s-tile st: out[s,:] = sum_h attn_h @ wo_h
            (fp8 chains over both head pairs); one wide row DMA per tile
            (narrow DMAs for the last tile - shorter shutdown chain)."""
            narrow = (st == NST - 1)
            orow = out_p.tile([128, D], f16, tag="outp")
            for eg in range(2):
                ops = [ps.tile([128, SQ], f32, tag="ps", name="oproj")[:]
                       for _ in range(2)]
                for g in range(2):
                    woh_t, wol_t = wo2[g]
                    for ci, (ca, cw) in enumerate(
                            [(a2h[g], woh_t), (a2l[g], woh_t),
                             (a2h[g], wol_t)]):
                        for ei in range(2):
                            e = 2 * eg + ei
                            nc.tensor.matmul(
                                ops[ei],
                                ca[:, :, ts(st, 128)],
                                cw[:, :, ts(e, SQ)],
                                start=(g == 0 and ci == 0),
                                stop=(g == 1 and ci == 2),
                                perf_mode=DR)
                for ei in range(2):
                    e = 2 * eg + ei
                    if ncopy[0] % 2 == 0:
                        nc.scalar.mul(orow[:, ts(e, SQ)], ops[ei],
                                      1.0 / 64.0)
                    else:
                        nc.vector.tensor_scalar_mul(orow[:, ts(e, SQ)],
                                                    ops[ei], 1.0 / 64.0)
                    ncopy[0] += 1
                    if narrow:
                        nc.sync.dma_start(
                            out=out_d[st * 128:(st + 1) * 128,
                                      e * SQ:(e + 1) * SQ],
                            in_=orow[:, ts(e, SQ)])
            if not narrow:
                nc.sync.dma_start(out=out_d[st * 128:(st + 1) * 128, :],
                                  in_=orow[:])

        w1_tiles = [None]

        for pair in range(2):
            w_tiles = (wh0, wl0) if pair == 0 else w1_tiles[0]

            # Q/K projection (+bias) and rope -> per-head contiguous tiles
            qT = [qk_p.tile([128, S], bf16, tag="qk", name="qT")
                  for _ in range(2)]
            kT = [qk_p.tile([128, S], bf16, tag="qk", name="kT")
                  for _ in range(2)]
            for jg in range(2):
                accs = qkv_pass(w_tiles, pair,
                                [(j, m) for j in (2 * jg, 2 * jg + 1)
                                 for m in range(4)])
                for j in (2 * jg, 2 * jg + 1):
                    rope(j, accs[(j, 0)], accs[(j, 1)], qT)
                    rope(j, accs[(j, 2)], accs[(j, 3)], kT)

            # V projection -> V^T -> transpose to [s, d] layout per head.
            # Two 4-acc passes: pass A's psum banks drain (copies + DMA-xbar
            # transposes) while pass B computes, so attention starts with
            # banks already free.
            vts = [vt_p.tile([128, S], bf16, tag="vt", name="vt0"),
                   vt_p.tile([128, S], bf16, tag="vt", name="vt1")]
            vs = [v_p.tile([128, NST, 128], bf16, tag="v", name="v0"),
                  v_p.tile([128, NST, 128], bf16, tag="v", name="v1")]
            for jg in range(2):
                vacc = qkv_pass(w_tiles, pair,
                                [(j, 4 + hh) for j in (2 * jg, 2 * jg + 1)
                                 for hh in range(2)])
                for ji, j in enumerate((2 * jg, 2 * jg + 1)):
                    for hh in range(2):
                        # alternate engines so the psum banks drain fast
                        if (ji + hh) % 2 == 0:
                            nc.scalar.mul(vts[hh][:, ts(j, SQ)],
                                          vacc[(j, 4 + hh)], 1.0 / 64.0)
                        else:
                            nc.vector.tensor_scalar_mul(
                                vts[hh][:, ts(j, SQ)],
                                vacc[(j, 4 + hh)], 1.0 / 64.0)
                        nc.sync.dma_start_transpose(
                            vs[hh][:, 4 * j:4 * j + 4, :],
                            vts[hh][:, ts(j, SQ)])

            if pair == 0:
                # pair-1 weights stream during pair-0 attention (DMA idle;
                # the w pool bufs free as pair-0''s last pass retires)
                w1_tiles[0] = load_pair_w(1)
            else:
                # wo streams during pair-1 attention
                for g in range(2):
                    th = wo_p.tile([128, 2, D], fp8, tag="wo", name="woh_t")
                    nc.sync.dma_start(out=th[:],
                                      in_=woh_d[g * 128:(g + 1) * 128, :])
                    tl = wo_p.tile([128, 2, D], fp8, tag="wo", name="wol_t")
                    nc.sync.dma_start(out=tl[:],
                                      in_=wol_d[g * 128:(g + 1) * 128, :])
                    wo2.append((th, tl))

            # attention for the pair's two heads, processed jointly.  For
            # pair 1, o_proj s-tile blocks are interleaved at j boundaries:
            # o_proj rows 512j..512j+511 need only chunks <= j of both pairs,
            # and the extra PE work fills the softmax-tail bubbles.
            pending = None
            for j in range(NJ):
                pending = attn_j(pair, j, qT, kT, vs, pending)
            # fire the last chunk's tail before the next phase needs psum
            pending()

        for st in range(NST):
            oproj_block(st)

    nc.compile()
    return nc


def _split8(a):
    hi = np.clip(a, -240.0, 240.0).astype(F8)
    lo = (a - hi.astype(np.float32)).astype(F8)
    return hi, lo


def _pairify(a):
    """[K, C] -> [K/2, 2C]: row kp*128+p holds k-chunks (2kp, 2kp+1) side
    by side (DoubleRow k-pair layout)."""
    Kd, C = a.shape
    return np.ascontiguousarray(
        a.reshape(Kd // 256, 2, 128, C).transpose(0, 2, 1, 3)
        .reshape(Kd // 2, 2 * C))


def _host_inputs(x, w_qkv, b_qkv, w_o):
    """Build the 8 per-core input maps."""
    x = np.asarray(x, dtype=np.float32)
    w_qkv = np.asarray(w_qkv, dtype=np.float32)
    b_qkv = np.asarray(b_qkv, dtype=np.float32)
    w_o = np.asarray(w_o, dtype=np.float32)

    # rope tables (reference swaps sin/cos roles; we follow the math:
    # q_rot = q*sin(emb) + rotate_half(q)*cos(emb)); 1/64 de-scales the
    # 64x weight scaling used to keep fp8 weight parts in normal range
    inv_freq = 1.0 / (10000.0 ** (np.arange(0, HD, 2, dtype=np.float32) / HD))
    t = np.arange(S, dtype=np.float32)
    freq = np.einsum("s,f->sf", t, inv_freq)          # [S, 64]
    sinT = np.sin(freq).T.astype(np.float32) / 64.0   # [64, S]
    cosT = np.cos(freq).T.astype(np.float32) / 64.0
    stab = np.concatenate([sinT, sinT], 0).astype(BF16)   # [128, S]
    ctab = np.concatenate([cosT, cosT], 0).astype(BF16)

    p_idx = np.arange(128)[:, None]
    f_idx = np.arange(128)[None, :]
    # used as matmul lhsT (accumulated as cmask.T @ ident into the logits):
    # effective additive mask[sk, sq] = cmask[sq, sk] = -9e15 where sq < sk
    cmask = np.where(p_idx >= f_idx, 0.0, -9e15).astype(BF16)
    ident = np.eye(128, dtype=np.float32).astype(BF16)

    def head_w(h):
        base = h * 3 * HD
        return (w_qkv[:, base:base + HD],
                w_qkv[:, base + HD:base + 2 * HD],
                w_qkv[:, base + 2 * HD:base + 3 * HD])

    def head_b(h):
        base = h * 3 * HD
        return (b_qkv[base:base + HD],
                b_qkv[base + HD:base + 2 * HD],
                b_qkv[base + 2 * HD:base + 3 * HD])

    in_maps = []
    for c in range(N_CORES):
        b = c // 4
        heads = [4 * (c % 4) + i for i in range(4)]
        xT = np.ascontiguousarray(x[b].T)               # [D, S] f32
        xh, xl = _split8(xT)
        xh = _pairify(xh)
        xl = _pairify(xl)

        mats, bvec = [], []
        for pair in range(2):
            ha, hb = heads[2 * pair], heads[2 * pair + 1]
            wq_a, wk_a, wv_a = head_w(ha)
            wq_b, wk_b, wv_b = head_w(hb)
            bq_a, bk_a, bv_a = head_b(ha)
            bq_b, bk_b, bv_b = head_b(hb)
            mats += [
                np.concatenate([wq_a[:, :64], wq_b[:, :64]], 1),
                np.concatenate([wq_a[:, 64:], wq_b[:, 64:]], 1),
                np.concatenate([wk_a[:, :64], wk_b[:, :64]], 1),
                np.concatenate([wk_a[:, 64:], wk_b[:, 64:]], 1),
                wv_a, wv_b,
            ]
            bvec += [
                np.concatenate([bq_a[:64], bq_b[:64]]),
                np.concatenate([bq_a[64:], bq_b[64:]]),
                np.concatenate([bk_a[:64], bk_b[:64]]),
                np.concatenate([bk_a[64:], bk_b[64:]]),
                bv_a, bv_b,
            ]
        w_all = np.concatenate(mats, 1) * 64.0               # [D, 1536]

        def _w_layout(a):
            # [2048, 1536] -> rows kp*128+p, cols pair*1536 + slot*768 + c
            return np.ascontiguousarray(
                a.reshape(KP, 2, 128, 2, 768).transpose(0, 2, 3, 1, 4)
                .reshape(1024, 3072))

        wh8, wl8 = _split8(w_all)
        wh8 = _w_layout(wh8)
        wl8 = _w_layout(wl8)
        bias_rows = np.zeros((12, 256), dtype=F8)
        for i, bv in enumerate(bvec):
            bh, bl = _split8(bv * 64.0)
            bias_rows[i, :128] = bh
            bias_rows[i, 128:] = bl

        wo_all = np.concatenate(
            [w_o[h * HD:(h + 1) * HD, :] for h in heads], 0) * 64.0  # [512,D]
        woh8, wol8 = _split8(wo_all)
        # [512, D] -> rows g*128+p, cols slot*D+e (slot = head-in-pair)
        woh8 = np.ascontiguousarray(
            woh8.reshape(2, 2, 128, D).transpose(0, 2, 1, 3).reshape(256, 2 * D))
        wol8 = np.ascontiguousarray(
            wol8.reshape(2, 2, 128, D).transpose(0, 2, 1, 3).reshape(256, 2 * D))

        in_maps.append({
            "xh": xh, "xl": xl, "wh": wh8, "wl": wl8, "bias": bias_rows,
            "woh": woh8, "wol": wol8,
            "stab": stab, "ctab": ctab, "cmask": cmask, "ident": ident,
        })
    return in_maps


def _run(in_maps, trace=False):
    from concourse.bass_utils import run_bass_kernel_spmd
    if "nc" not in _MODULE_CACHE:
        _MODULE_CACHE["nc"] = _build_module()
    nc = _MODULE_CACHE["nc"]
    return run_bass_kernel_spmd(nc, in_maps, core_ids=list(range(N_CORES)),
                                trace=trace)


def kernel(x, w_qkv, b_qkv, w_o, b_o, _trace=False, _return_res=False):
    in_maps = _host_inputs(x, w_qkv, b_qkv, w_o)
    res = _run(in_maps, trace=_trace)
    out = np.zeros((2, S, D), dtype=np.float32)
    for c in range(N_CORES):
        out[c // 4] += res.results[c]["out"].astype(np.float32)
    out += np.asarray(b_o, dtype=np.float32)[None, None, :]
    if _return_res:
        return out, res
    return out


# revision 67
# speedup vs baseline: 1.0473x; 1.0091x over previous
"""Trainium2 Bass kernel for fused multi-head causal attention.

Module: out = o_proj(causal_attention(rope_swapped(qkv_proj(x)))).
Shapes: x [2, 2048, 2048], 16 heads, head_dim 128.

Sharding (8 cores): batch (2) x head-group (4 groups of 4 heads).
Each core computes qkv projection + attention for its 4 heads of its
batch, then a partial o_proj against its slice of w_o rows.  The
all-reduce after o_proj is done host-side by summing the 4 partials
per batch (mathematically identical, avoids device collectives).

Device-kernel design notes:
 - QKV and o_proj matmuls run in fp8-e4m3 DoubleRow perf mode (0.5
   cycles/row on PE) with hi+lo error compensation: a ~= hi(a) + lo(a)
   with both parts fp8; a@b ~= ah@bh + al@bh + ah@bl (lo x lo dropped,
   ~0.1% error).  3 chains x half-rate = 0.75x the bf16 PE cost.
   Weights are scaled by 64 host-side so their fp8 parts stay normal;
   the 1/64 is folded into the rope tables / V copy / output copy.
 - DoubleRow contracts 2 k-chunks of 128 per instruction; operands are
   laid out [128 partitions, 2, free] with the paired k-chunks adjacent
   in the free dim (prepared host-side).
 - The qkv bias is added with a single fp8 DoubleRow rank-1 matmul per
   output tile: lhsT [1, 2, 128] = (bias_hi | bias_lo), rhs = ones.
 - QKV projection computes Q^T/K^T/V^T ([head_dim, S] layout) directly:
   out = W_slice.T @ x^T, so attention's QK^T matmul needs no transposes.
 - Q/K head pairs are interleaved ([q_ha_lo | q_hb_lo] on 128 partitions)
   so RoPE's rotate_half partner lives at the SAME partition of a sibling
   tile -> full-width DVE ops (3 ops/element, no cross-partition shuffle).
 - Scores are computed transposed ([sk, sq]) so the PV matmul consumes
   exp(scores) directly; softmax denominators come from a ones-vector
   matmul; normalization is folded into the PSUM->SBUF copy of attn^T,
   which also emits the fp8 hi/lo pair layout for the o_proj chains.
 - Causal masking: fully-masked columns of diagonal score tiles are
   skipped outright; the remaining width-128 triangle is accumulated
   into the logits psum ON THE PE (cmask^T @ ident) so no vector-engine
   hop sits between QK^T and the exp.
 - V^T -> V transposes go through the DMA xbar (dma_start_transpose),
   off the PE/psum entirely; V projection runs as two 4-acc passes so
   its psum banks drain while the second pass computes.
 - Attention core (QK^T, exp, PV, denominators) stays bf16: a single
   uncompensated fp8 tensor there would inject ~3.6% output error
   (the hi/lo compensation only pays at contraction >= 256).
 - Output rows are assembled [128, 2048] f16 in SBUF and written with
   one wide DMA per s-tile (halves output bytes, 4x fewer DMAs);
   partials are summed in fp32 host-side.
"""

import math

import ml_dtypes
import numpy as np

S = 2048
D = 2048
HD = 128
NH = 16
N_CORES = 8
SQ = 512          # free-dim chunk for matmuls / psum tiles
NJ = S // SQ      # 4 s-chunks
KP = D // 256     # 8 contraction k-pair chunks (DoubleRow: 256 each)
NST = S // 128    # 16 s-tiles of 128
BF16 = ml_dtypes.bfloat16
F16 = np.float16
F8 = ml_dtypes.float8_e4m3

_MODULE_CACHE = {}


def _build_module():
    from contextlib import ExitStack

    import concourse.bass as bass
    import concourse.bacc as bacc
    import concourse.mybir as mybir
    import concourse.tile as tile

    f32 = mybir.dt.float32
    f16 = mybir.dt.float16
    bf16 = mybir.dt.bfloat16
    fp8 = mybir.dt.float8e4
    DR = mybir.MatmulPerfMode.DoubleRow
    ts = bass.ts

    nc = bacc.Bacc("TRN2", target_bir_lowering=False, debug=False,
                   num_devices=N_CORES)

    # DRAM I/O (identical program on all cores; per-core data differs)
    xh_d = nc.dram_tensor("xh", [1024, 2 * S], fp8, kind="ExternalInput").ap()
    xl_d = nc.dram_tensor("xl", [1024, 2 * S], fp8, kind="ExternalInput").ap()
    wh_d = nc.dram_tensor("wh", [1024, 2, 2, 768], fp8,
                          kind="ExternalInput").ap()
    wl_d = nc.dram_tensor("wl", [1024, 2, 2, 768], fp8,
                          kind="ExternalInput").ap()
    bias_d = nc.dram_tensor("bias", [12, 256], fp8, kind="ExternalInput").ap()
    woh_d = nc.dram_tensor("woh", [256, 2 * D], fp8, kind="ExternalInput").ap()
    wol_d = nc.dram_tensor("wol", [256, 2 * D], fp8, kind="ExternalInput").ap()
    stab_d = nc.dram_tensor("stab", [128, S], bf16, kind="ExternalInput").ap()
    ctab_d = nc.dram_tensor("ctab", [128, S], bf16, kind="ExternalInput").ap()
    cmask_d = nc.dram_tensor("cmask", [128, 128], bf16, kind="ExternalInput").ap()
    ident_d = nc.dram_tensor("ident", [128, 128], bf16, kind="ExternalInput").ap()
    out_d = nc.dram_tensor("out", [S, D], f16, kind="ExternalOutput").ap()

    inv_sqrt_hd = 1.0 / math.sqrt(HD)

    with tile.TileContext(nc) as tc, ExitStack() as ctx:
        # Long-lived pools first; x/w/tab/vt live in an inner scope that is
        # closed after the last QKV pass so the wo pool can reuse the space
        # (SBUF pool allocation is a LIFO stack).
        ps = ctx.enter_context(
            tc.tile_pool(name="ps", bufs=8, space=bass.MemorySpace.PSUM))
        const_p = ctx.enter_context(tc.tile_pool(name="const", bufs=1))
        qk_p = ctx.enter_context(tc.tile_pool(name="qk", bufs=4))
        v_p = ctx.enter_context(tc.tile_pool(name="v", bufs=2))
        scr_p = ctx.enter_context(tc.tile_pool(name="scr", bufs=10))
        p_p = ctx.enter_context(tc.tile_pool(name="p", bufs=6))
        attn_p = ctx.enter_context(tc.tile_pool(name="attn", bufs=4))
        af_p = ctx.enter_context(tc.tile_pool(name="af", bufs=3))
        bc_p = ctx.enter_context(tc.tile_pool(name="bc", bufs=2))
        rcp_p = ctx.enter_context(tc.tile_pool(name="rcp", bufs=2))
        out_p = ctx.enter_context(tc.tile_pool(name="outp", bufs=3))
        wo_p = ctx.enter_context(tc.tile_pool(name="wo", bufs=4))
        ctx2 = ctx.enter_context(ExitStack())
        x_p = ctx2.enter_context(tc.tile_pool(name="xp", bufs=16))
        w_p = ctx2.enter_context(tc.tile_pool(name="wp", bufs=16))
        tab_p = ctx2.enter_context(tc.tile_pool(name="tab", bufs=2))
        vt_p = ctx2.enter_context(tc.tile_pool(name="vt", bufs=2))

        # x^T (hi+lo fp8, k-pair layout) resident in SBUF; pair-0 weights
        # interleaved so the first QKV pass starts immediately and the
        # stream stays just ahead of PE consumption.
        # the first QK pass reads only w's m-slices 0..3 (cols 0:512 per
        # slot): the V columns (512:768) are deferred out of the critical
        # pass-0 stream and land during the QK passes
        xh_t, xl_t = [], []
        wh0, wl0 = [], []
        for kp in range(KP):
            t = x_p.tile([128, 2, S], fp8, tag="x", name="xh")
            nc.sync.dma_start(out=t[:], in_=xh_d[kp * 128:(kp + 1) * 128, :])
            xh_t.append(t)
            t = w_p.tile([128, 2, 768], fp8, tag="w", name="wh0")
            nc.sync.dma_start(out=t[:, :, 0:512],
                              in_=wh_d[kp * 128:(kp + 1) * 128, 0, :, 0:512])
            wh0.append(t)
            t = x_p.tile([128, 2, S], fp8, tag="x", name="xl")
            nc.sync.dma_start(out=t[:], in_=xl_d[kp * 128:(kp + 1) * 128, :])
            xl_t.append(t)
            t = w_p.tile([128, 2, 768], fp8, tag="w", name="wl0")
            nc.sync.dma_start(out=t[:, :, 0:512],
                              in_=wl_d[kp * 128:(kp + 1) * 128, 0, :, 0:512])
            wl0.append(t)
        # qkv bias first (pass jg=0's stop matmuls need it earliest),
        # then rope tables, then the deferred w V-columns (used from the
        # V passes at ~45us), then attention constants
        bias_t = []
        for i in range(12):
            t = const_p.tile([1, 2, 128], fp8, tag=f"b{i}")
            nc.sync.dma_start(out=t[:], in_=bias_d[i:i + 1, :])
            bias_t.append(t)
        stab = tab_p.tile([128, S], bf16, tag="tab")
        ctab = tab_p.tile([128, S], bf16, tag="tab")
        # first-half columns land first: rope j=0,1 unblocks ~1.6us earlier
        nc.sync.dma_start(out=stab[:, 0:1024], in_=stab_d[:, 0:1024])
        nc.sync.dma_start(out=ctab[:, 0:1024], in_=ctab_d[:, 0:1024])
        nc.sync.dma_start(out=stab[:, 1024:S], in_=stab_d[:, 1024:S])
        nc.sync.dma_start(out=ctab[:, 1024:S], in_=ctab_d[:, 1024:S])
        for kp in range(KP):
            nc.sync.dma_start(out=wh0[kp][:, :, 512:768],
                              in_=wh_d[kp * 128:(kp + 1) * 128, 0, :,
                                       512:768])
            nc.sync.dma_start(out=wl0[kp][:, :, 512:768],
                              in_=wl_d[kp * 128:(kp + 1) * 128, 0, :,
                                       512:768])
        cmask = const_p.tile([128, 128], bf16, tag="c0")
        nc.sync.dma_start(out=cmask[:], in_=cmask_d[:])
        ident = const_p.tile([128, 128], bf16, tag="c1")
        nc.sync.dma_start(out=ident[:], in_=ident_d[:])
        ones2 = const_p.tile([1, 2, SQ], fp8, tag="c4")
        nc.vector.memset(ones2[:], 1.0)
        ones_col = const_p.tile([128, 1], bf16, tag="c5")
        nc.vector.memset(ones_col[:], 1.0)
        ones_f16 = const_p.tile([1, 128], f16, tag="c6")
        nc.vector.memset(ones_f16[:], 1.0)

        def load_pair_w(pair):
            whs, wls = [], []
            for kp in range(KP):
                t = w_p.tile([128, 2, 768], fp8, tag="w", name="wh1")
                nc.sync.dma_start(
                    out=t[:], in_=wh_d[kp * 128:(kp + 1) * 128, pair, :, :])
                whs.append(t)
                t = w_p.tile([128, 2, 768], fp8, tag="w", name="wl1")
                nc.sync.dma_start(
                    out=t[:], in_=wl_d[kp * 128:(kp + 1) * 128, pair, :, :])
                wls.append(t)
            return whs, wls

        def qkv_pass(w_tiles, pair, jms):
            """Accumulate 64*(x @ W + b) for the given (j, mat) pairs via
            3 fp8 DoubleRow chains.  jms must be 8 accs forming 4 rope/V
            pairs (consecutive); 3 pairs land in 2-bank psb tiles, the 4th
            in two pss tiles.  Returns {(j, m): psum AP [128, SQ] f32}."""
            whs, wls = w_tiles
            accs = {}
            for jm in jms:
                accs[jm] = ps.tile([128, SQ], f32, tag="ps",
                                   name="qkv_acc")[:]
            for kp in range(KP):
                for ci, (cx, cw) in enumerate(
                        [(xh_t, whs), (xl_t, whs), (xh_t, wls)]):
                    for (j, m) in jms:
                        nc.tensor.matmul(
                            accs[(j, m)],
                            cw[kp][:, :, ts(m, 128)],
                            cx[kp][:, :, ts(j, SQ)],
                            start=(kp == 0 and ci == 0), stop=False,
                            perf_mode=DR)
            for (j, m) in jms:
                nc.tensor.matmul(
                    accs[(j, m)],
                    bias_t[pair * 6 + m][:, :, :],
                    ones2[:, :, :],
                    start=False, stop=True, perf_mode=DR)
            return accs

        def rope(j, A, B, dsts):
            """A=[lo ha|lo hb], B=[hi ha|hi hb] pair-interleaved psum tiles
            (64x scale); writes per-head contiguous rotated [128, SQ] slices
            into dsts[0] (head a) and dsts[1] (head b).  Tables carry the
            1/64:  rot_lo = lo*sin - hi*cos ; rot_hi = hi*sin + lo*cos."""
            sl = stab[:, ts(j, SQ)]
            cl = ctab[:, ts(j, SQ)]
            # drain psum -> bf16 SBUF on the scalar engine first: frees the
            # psum bank after 2 ops (not 4) and makes every DVE op below
            # all-bf16/SBUF -> 2x DVE rate
            A2 = scr_p.tile([128, SQ], bf16, tag="scr")
            nc.scalar.copy(A2[:], A)
            B2 = scr_p.tile([128, SQ], bf16, tag="scr")
            nc.scalar.copy(B2[:], B)
            t1 = scr_p.tile([128, SQ], bf16, tag="scr")
            nc.vector.tensor_mul(t1[:], A2[:], sl)
            t2 = scr_p.tile([128, SQ], bf16, tag="scr")
            nc.vector.tensor_mul(t2[:], B2[:], cl)
            t3 = scr_p.tile([128, SQ], bf16, tag="scr")
            nc.vector.tensor_mul(t3[:], B2[:], sl)
            t4 = scr_p.tile([128, SQ], bf16, tag="scr")
            nc.vector.tensor_mul(t4[:], A2[:], cl)
            for hh in range(2):
                hs = slice(64 * hh, 64 * hh + 64)
                nc.vector.tensor_sub(dsts[hh][0:64, ts(j, SQ)],
                                     t1[hs, :], t2[hs, :])
                nc.vector.tensor_add(dsts[hh][64:128, ts(j, SQ)],
                                     t3[hs, :], t4[hs, :])

        # attn pair tiles for o_proj fp8 chains: [128, 2, S], slot = head
        a2h = [attn_p.tile([128, 2, S], fp8, tag="attn", name="a2h")
               for _ in range(2)]
        a2l = [attn_p.tile([128, 2, S], fp8, tag="attn", name="a2l")
               for _ in range(2)]

        def attn_j(pair, j, qT, kT, vs, pending=None):
            """Emit attention for s-chunk j of the given head pair.  The
            previous chunk's softmax tail (broadcast matmuls etc.) is
            emitted AFTER this chunk's first two i-steps so the in-order
            PE stream is never head-of-line blocked waiting on the
            reciprocal; returns this chunk's own tail closure."""
            ndiag = 4 * j + 4
            apsum = dpsum = None
            for i in range(ndiag):
                r = i - 4 * j
                # columns left of a diagonal tile's valid triangle are
                # fully masked -> skip them in QK/exp/PV/denom entirely
                off = 128 * r if r > 0 else 0
                lg = [ps.tile([128, SQ], f32, tag="ps", name="lg")
                      for _ in range(2)]
                # logits^T[sk,sq] = K^T.T @ Q^T; on diagonal tiles the
                # causal -9e15 triangle is accumulated on the PE itself
                # (cmask^T @ ident, 128 cols) - no DVE hop before exp
                for hh in range(2):
                    nc.tensor.matmul(lg[hh][:, off:SQ],
                                     kT[hh][:, ts(i, 128)],
                                     qT[hh][:, j * SQ + off:(j + 1) * SQ],
                                     start=True, stop=(r < 0))
                    if r >= 0:
                        nc.tensor.matmul(lg[hh][:, off:off + 128],
                                         cmask[:], ident[:],
                                         start=False, stop=True)
                if i == 0:
                    # allocated after the first logits tiles so the next
                    # chunk's QK matmuls can start while the previous
                    # chunk is still normalizing
                    apsum = [ps.tile([128, SQ], f32, tag="ps",
                                     name="apsum")[:] for _ in range(2)]
                    dpsum = ps.tile([33, SQ], f32, tag="ps",
                                    name="dpsum")
                last = (i == ndiag - 1)
                for hh in range(2):
                    p_t = p_p.tile([128, SQ], bf16, tag="p")
                    nc.scalar.activation(
                        p_t[:, off:SQ], lg[hh][:, off:SQ],
                        mybir.ActivationFunctionType.Exp,
                        scale=inv_sqrt_hd)
                    nc.tensor.matmul(dpsum[32 * hh:32 * hh + 1, off:SQ],
                                     ones_col[:, 0:1],
                                     p_t[:, off:SQ],
                                     start=(i == 0), stop=last)
                    nc.tensor.matmul(apsum[hh][:, off:SQ],
                                     vs[hh][:, i, :],
                                     p_t[:, off:SQ],
                                     start=(i == 0), stop=last)
                if i == 1 and pending is not None:
                    pending()
                    pending = None
            if pending is not None:
                pending()

            # recips issue now (DVE, frees dpsum); the PE-side broadcast
            # and the normalize run from the deferred closure
            rcs = []
            for hh in range(2):
                # fp16 reciprocal: full-rate matmul dtype, 2^-11 relative
                # precision is ample for softmax denominators
                rc = rcp_p.tile([1, SQ], f16, tag="rcp")
                with nc.allow_low_precision(reason="fp16 1/denom"):
                    nc.vector.reciprocal(rc[:],
                                         dpsum[32 * hh:32 * hh + 1, :])
                rcs.append(rc)
            my_apsum = apsum

            def tail():
                bcps, bcs = [], []
                for hh in range(2):
                    # broadcast 1/denom across partitions via a K=1 matmul
                    bcp = ps.tile([128, SQ], f32, tag="ps", name="bcp")
                    nc.tensor.matmul(bcp[:], ones_f16[0:1, :],
                                     rcs[hh][0:1, :],
                                     start=True, stop=True)
                    bcps.append(bcp)
                for hh in range(2):
                    bc = bc_p.tile([128, SQ], f32, tag="bc")
                    nc.vector.tensor_copy(bc[:], bcps[hh][:])
                    bcs.append(bc)
                for hh in range(2):
                    # normalized attn^T -> f16, then fp8 hi/lo pair layout
                    af = af_p.tile([128, SQ], f16, tag="af")
                    nc.vector.tensor_mul(af[:], my_apsum[hh], bcs[hh][:])
                    nc.gpsimd.tensor_copy(a2h[pair][:, hh, ts(j, SQ)],
                                          af[:])
                    nc.gpsimd.tensor_sub(a2l[pair][:, hh, ts(j, SQ)],
                                         af[:],
                                         a2h[pair][:, hh, ts(j, SQ)])
            return tail

        ncopy = [0]
        wo2 = []

        def oproj_block(st):
            """o_proj partial for s-tile st: out[s,:] = sum_h attn_h @ wo_h
            (fp8 chains over both head pairs); one wide row DMA per tile
            (narrow DMAs for the last tile - shorter shutdown chain)."""
            narrow = (st == NST - 1)
            orow = out_p.tile([128, D], f16, tag="outp")
            for eg in range(2):
                ops = [ps.tile([128, SQ], f32, tag="ps", name="oproj")[:]
                       for _ in range(2)]
                for g in range(2):
                    woh_t, wol_t = wo2[g]
                    for ci, (ca, cw) in enumerate(
                            [(a2h[g], woh_t), (a2l[g], woh_t),
                             (a2h[g], wol_t)]):
                        for ei in range(2):
                            e = 2 * eg + ei
                            nc.tensor.matmul(
                                ops[ei],
                                ca[:, :, ts(st, 128)],
                                cw[:, :, ts(e, SQ)],
                                start=(g == 0 and ci == 0),
                                stop=(g == 1 and ci == 2),
                                perf_mode=DR)
                for ei in range(2):
                    e = 2 * eg + ei
                    if ncopy[0] % 2 == 0:
                        nc.scalar.mul(orow[:, ts(e, SQ)], ops[ei],
                                      1.0 / 64.0)
                    else:
                        nc.vector.tensor_scalar_mul(orow[:, ts(e, SQ)],
                                                    ops[ei], 1.0 / 64.0)
                    ncopy[0] += 1
                    if narrow:
                        nc.sync.dma_start(
                            out=out_d[st * 128:(st + 1) * 128,
                                      e * SQ:(e + 1) * SQ],
                            in_=orow[:, ts(e, SQ)])
            if not narrow:
                nc.sync.dma_start(out=out_d[st * 128:(st + 1) * 128, :],
                                  in_=orow[:])

        w1_tiles = [None]

        for pair in range(2):
            w_tiles = (wh0, wl0) if pair == 0 else w1_tiles[0]

            # Q/K projection (+bias) and rope -> per-head contiguous tiles
            qT = [qk_p.tile([128, S], bf16, tag="qk", name="qT")
                  for _ in range(2)]
            kT = [qk_p.tile([128, S], bf16, tag="qk", name="kT")
                  for _ in range(2)]
            for jg in range(2):
                accs = qkv_pass(w_tiles, pair,
                                [(j, m) for j in (2 * jg, 2 * jg + 1)
                                 for m in range(4)])
                for j in (2 * jg, 2 * jg + 1):
                    rope(j, accs[(j, 0)], accs[(j, 1)], qT)
                    rope(j, accs[(j, 2)], accs[(j, 3)], kT)

            # V projection -> V^T -> transpose to [s, d] layout per head.
            # Two 4-acc passes: pass A's psum banks drain (copies + DMA-xbar
            # transposes) while pass B computes, so attention starts with
            # banks already free.
            vts = [vt_p.tile([128, S], bf16, tag="vt", name="vt0"),
                   vt_p.tile([128, S], bf16, tag="vt", name="vt1")]
            vs = [v_p.tile([128, NST, 128], bf16, tag="v", name="v0"),
                  v_p.tile([128, NST, 128], bf16, tag="v", name="v1")]
            for jg in range(2):
                vacc = qkv_pass(w_tiles, pair,
                                [(j, 4 + hh) for j in (2 * jg, 2 * jg + 1)
                                 for hh in range(2)])
                for ji, j in enumerate((2 * jg, 2 * jg + 1)):
                    for hh in range(2):
                        # alternate engines so the psum banks drain fast
                        if (ji + hh) % 2 == 0:
                            nc.scalar.mul(vts[hh][:, ts(j, SQ)],
                                          vacc[(j, 4 + hh)], 1.0 / 64.0)
                        else:
                            nc.vector.tensor_scalar_mul(
                                vts[hh][:, ts(j, SQ)],
                                vacc[(j, 4 + hh)], 1.0 / 64.0)
                        nc.sync.dma_start_transpose(
                            vs[hh][:, 4 * j:4 * j + 4, :],
                            vts[hh][:, ts(j, SQ)])

            if pair == 0:
                # pair-1 weights stream during pair-0 attention (DMA idle;
                # the w pool bufs free as pair-0''s last pass retires)
                w1_tiles[0] = load_pair_w(1)
            else:
                # wo streams during pair-1 attention
                for g in range(2):
                    th = wo_p.tile([128, 2, D], fp8, tag="wo", name="woh_t")
                    nc.sync.dma_start(out=th[:],
                                      in_=woh_d[g * 128:(g + 1) * 128, :])
                    tl = wo_p.tile([128, 2, D], fp8, tag="wo", name="wol_t")
                    nc.sync.dma_start(out=tl[:],
                                      in_=wol_d[g * 128:(g + 1) * 128, :])
                    wo2.append((th, tl))

            # attention for the pair's two heads, processed jointly.  For
            # pair 1, o_proj s-tile blocks are interleaved at j boundaries:
            # o_proj rows 512j..512j+511 need only chunks <= j of both pairs,
            # and the extra PE work fills the softmax-tail bubbles.
            pending = None
            for j in range(NJ):
                pending = attn_j(pair, j, qT, kT, vs, pending)
            # fire the last chunk's tail before the next phase needs psum
            pending()

        for st in range(NST):
            oproj_block(st)

    nc.compile()
    return nc


def _split8(a):
    hi = np.clip(a, -240.0, 240.0).astype(F8)
    lo = (a - hi.astype(np.float32)).astype(F8)
    return hi, lo


def _pairify(a):
    """[K, C] -> [K/2, 2C]: row kp*128+p holds k-chunks (2kp, 2kp+1) side
    by side (DoubleRow k-pair layout)."""
    Kd, C = a.shape
    return np.ascontiguousarray(
        a.reshape(Kd // 256, 2, 128, C).transpose(0, 2, 1, 3)
        .reshape(Kd // 2, 2 * C))


def _host_inputs(x, w_qkv, b_qkv, w_o):
    """Build the 8 per-core input maps."""
    x = np.asarray(x, dtype=np.float32)
    w_qkv = np.asarray(w_qkv, dtype=np.float32)
    b_qkv = np.asarray(b_qkv, dtype=np.float32)
    w_o = np.asarray(w_o, dtype=np.float32)

    # rope tables (reference swaps sin/cos roles; we follow the math:
    # q_rot = q*sin(emb) + rotate_half(q)*cos(emb)); 1/64 de-scales the
    # 64x weight scaling used to keep fp8 weight parts in normal range
    inv_freq = 1.0 / (10000.0 ** (np.arange(0, HD, 2, dtype=np.float32) / HD))
    t = np.arange(S, dtype=np.float32)
    freq = np.einsum("s,f->sf", t, inv_freq)          # [S, 64]
    sinT = np.sin(freq).T.astype(np.float32) / 64.0   # [64, S]
    cosT = np.cos(freq).T.astype(np.float32) / 64.0
    stab = np.concatenate([sinT, sinT], 0).astype(BF16)   # [128, S]
    ctab = np.concatenate([cosT, cosT], 0).astype(BF16)

    p_idx = np.arange(128)[:, None]
    f_idx = np.arange(128)[None, :]
    # used as matmul lhsT (accumulated as cmask.T @ ident into the logits):
    # effective additive mask[sk, sq] = cmask[sq, sk] = -9e15 where sq < sk
    cmask = np.where(p_idx >= f_idx, 0.0, -9e15).astype(BF16)
    ident = np.eye(128, dtype=np.float32).astype(BF16)

    def head_w(h):
        base = h * 3 * HD
        return (w_qkv[:, base:base + HD],
                w_qkv[:, base + HD:base + 2 * HD],
                w_qkv[:, base + 2 * HD:base + 3 * HD])

    def head_b(h):
        base = h * 3 * HD
        return (b_qkv[base:base + HD],
                b_qkv[base + HD:base + 2 * HD],
                b_qkv[base + 2 * HD:base + 3 * HD])

    in_maps = []
    for c in range(N_CORES):
        b = c // 4
        heads = [4 * (c % 4) + i for i in range(4)]
        xT = np.ascontiguousarray(x[b].T)               # [D, S] f32
        xh, xl = _split8(xT)
        xh = _pairify(xh)
        xl = _pairify(xl)

        mats, bvec = [], []
        for pair in range(2):
            ha, hb = heads[2 * pair], heads[2 * pair + 1]
            wq_a, wk_a, wv_a = head_w(ha)
            wq_b, wk_b, wv_b = head_w(hb)
            bq_a, bk_a, bv_a = head_b(ha)
            bq_b, bk_b, bv_b = head_b(hb)
            mats += [
                np.concatenate([wq_a[:, :64], wq_b[:, :64]], 1),
                np.concatenate([wq_a[:, 64:], wq_b[:, 64:]], 1),
                np.concatenate([wk_a[:, :64], wk_b[:, :64]], 1),
                np.concatenate([wk_a[:, 64:], wk_b[:, 64:]], 1),
                wv_a, wv_b,
            ]
            bvec += [
                np.concatenate([bq_a[:64], bq_b[:64]]),
                np.concatenate([bq_a[64:], bq_b[64:]]),
                np.concatenate([bk_a[:64], bk_b[:64]]),
                np.concatenate([bk_a[64:], bk_b[64:]]),
                bv_a, bv_b,
            ]
        w_all = np.concatenate(mats, 1) * 64.0               # [D, 1536]

        def _w_layout(a):
            # [2048, 1536] -> rows kp*128+p, cols pair*1536 + slot*768 + c
            return np.ascontiguousarray(
                a.reshape(KP, 2, 128, 2, 768).transpose(0, 2, 3, 1, 4)
                .reshape(1024, 3072))

        wh8, wl8 = _split8(w_all)
        wh8 = _w_layout(wh8)
        wl8 = _w_layout(wl8)
        bias_rows = np.zeros((12, 256), dtype=F8)
        for i, bv in enumerate(bvec):
            bh, bl = _split8(bv * 64.0)
            bias_rows[i, :128] = bh
            bias_rows[i, 128:] = bl

        wo_all = np.concatenate(
            [w_o[h * HD:(h + 1) * HD, :] for h in heads], 0) * 64.0  # [512,D]
        woh8, wol8 = _split8(wo_all)
        # [512, D] -> rows g*128+p, cols slot*D+e (slot = head-in-pair)
        woh8 = np.ascontiguousarray(
            woh8.reshape(2, 2, 128, D).transpose(0, 2, 1, 3).reshape(256, 2 * D))
        wol8 = np.ascontiguousarray(
            wol8.reshape(2, 2, 128, D).transpose(0, 2, 1, 3).reshape(256, 2 * D))

        in_maps.append({
            "xh": xh, "xl": xl, "wh": wh8, "wl": wl8, "bias": bias_rows,
            "woh": woh8, "wol": wol8,
            "stab": stab, "ctab": ctab, "cmask": cmask, "ident": ident,
        })
    return in_maps


def _run(in_maps, trace=False):
    from concourse.bass_utils import run_bass_kernel_spmd
    if "nc" not in _MODULE_CACHE:
        _MODULE_CACHE["nc"] = _build_module()
    nc = _MODULE_CACHE["nc"]
    return run_bass_kernel_spmd(nc, in_maps, core_ids=list(range(N_CORES)),
                                trace=trace)


def kernel(x, w_qkv, b_qkv, w_o, b_o, _trace=False, _return_res=False):
    in_maps = _host_inputs(x, w_qkv, b_qkv, w_o)
    res = _run(in_maps, trace=_trace)
    out = np.zeros((2, S, D), dtype=np.float32)
    for c in range(N_CORES):
        out[c // 4] += res.results[c]["out"].astype(np.float32)
    out += np.asarray(b_o, dtype=np.float32)[None, None, :]
    if _return_res:
        return out, res
    return out
